# revision 8
# baseline (speedup 1.0000x reference)
"""Bayesian linear layer (per-sample weights) on 8 Trainium2 NeuronCores.

out[b,o] = sum_i x[b,i] * (eps[b,i,o]*softplus(ro)[i,o] + mu[i,o])
           + eps_bias[b,o]*softplus(ro_bias)[o] + mu_bias[o]

Strategy (2D sharding: 4 batch-groups x 2 i-halves per core):
  - Each core handles 32 samples and 512 of the 1024 contraction rows,
    producing a partial sum; the host unshard adds the two i-halves.
  - The binding resource is HBM read bandwidth. eps/ro/mu/x are
    converted to fp16 on the host (the 2e-2 rel-err budget is ~100x
    larger than fp16 rounding here), halving the streamed bytes:
    eps is 32MB/core instead of 64MB.
  - eps streams as 15 two-sample tiles + 1 single + 4 quarter-tiles,
    all on the sync HWDGE ring, which carries nothing else (a single
    queue keeps consecutive descriptors HBM-local; multi-queue
    interleaving measurably degrades per-descriptor service time).
    Params ride the scalar ring, as do the output stores.
  - Contraction rows are mapped i = c2*256 + 2p + c1 so every
    per-partition DMA run is 2 rows = 4KB contiguous (the descriptor
    size the DMA engines served fastest in traces).
  - DVE multiplies eps tiles by softplus(ro) in fp16 (2x DVE rate);
    TensorE consumes fp16 at full rate, reducing over i with M=1
    matmuls (lhsT = x column) into a [1,1024] f32 PSUM tile per
    sample; a one-hot K=32 matmul folds in the bias row (x@mu_half +
    bias terms on the j=0 core; zeros on j=1), the scalar engine
    copies PSUM -> SBUF and stores via its ring.
"""

import numpy as np

import concourse.bass as bass
import concourse.bacc as bacc
import concourse.mybir as mybir
from concourse.masks import make_identity
from concourse.tile import TileContext
from concourse.bass_utils import run_bass_kernel_spmd

F32 = mybir.dt.float32
F32R = mybir.dt.float32r
F16 = mybir.dt.float16
AF = mybir.ActivationFunctionType

B, IN, OUT = 128, 1024, 1024
NCORES = 8
BG = 4                    # batch groups
ISH = NCORES // BG        # i-shards (2)
BS = B // BG              # 32 samples per core
INS = IN // ISH           # 512 contraction rows per core
P = 128
CPP = INS // P            # 4 contraction rows per partition
FREE = CPP * OUT          # 4096 free elems per sample
HREE = FREE // 2


def build_nc():
    nc = bacc.Bacc(None, target_bir_lowering=False)

    eps_d = nc.declare_dram_parameter("eps", [BS, INS, OUT], F16, isOutput=False)
    ro_d = nc.declare_dram_parameter("ro", [INS, OUT], F16, isOutput=False)
    mu_d = nc.declare_dram_parameter("mu", [INS, OUT], F16, isOutput=False)
    # xt[p, cb*BS + b] = x[b, ishard*512 + c2*256 + 2p + c1], cb = 2*c2+c1
    xt_d = nc.declare_dram_parameter("xt", [P, CPP * BS], F16, isOutput=False)
    eb_d = nc.declare_dram_parameter("eps_bias", [BS, OUT], F32, isOutput=False)
    rb_d = nc.declare_dram_parameter("ro_bias", [BS, OUT], F32, isOutput=False)
    mb_d = nc.declare_dram_parameter("mu_bias", [BS, OUT], F32, isOutput=False)
    out_d = nc.declare_dram_parameter("out", [BS, OUT], F32, isOutput=True)

    # i_local = c2*256 + 2p + c1: per-partition DMA runs are 4KB in fp16
    ro_r = ro_d.rearrange("(c2 p c1) o -> p c2 c1 o", p=P, c1=2)
    mu_r = mu_d.rearrange("(c2 p c1) o -> p c2 c1 o", p=P, c1=2)

    with TileContext(nc) as tc:
        with (
            tc.tile_pool(name="const", bufs=1) as cpool,
            tc.tile_pool(name="eps", bufs=4) as epool,
            tc.tile_pool(name="elast", bufs=2) as lpool,
            tc.tile_pool(name="epr", bufs=4) as eprpool,
            tc.tile_pool(name="orow", bufs=3) as spool,
            tc.tile_pool(name="psmu", bufs=1, space="PSUM") as pmupool,
            tc.tile_pool(name="psum", bufs=3, space="PSUM") as ppool,
        ):
            # ---- softplus(ro) in quarters; f32 intermediate -------------
            sig = cpool.tile([P, FREE], F16)
            scr = cpool.tile([P, OUT], F32)
            for h in range(CPP):
                sl = sig[:, h * OUT : (h + 1) * OUT]
                nc.scalar.dma_start(out=sl, in_=ro_r[:, h // 2 : h // 2 + 1, h % 2 : h % 2 + 1, :])
                nc.scalar.activation(scr, sl, AF.Exp)
                nc.scalar.activation(sl, scr, AF.Ln, bias=1.0)

            xt = cpool.tile([P, CPP * BS], F16)
            nc.scalar.dma_start(out=xt, in_=xt_d[:, :])

            ident = cpool.tile([BS, BS], F32)
            make_identity(nc, ident)
            idr = cpool.tile([BS, BS], F32R)
            nc.vector.tensor_copy(out=idr, in_=ident)

            # ---- param loads (scalar ring; consumed lazily below) -------
            mt = cpool.tile([P, FREE], F16)
            nc.scalar.dma_start(out=mt, in_=mu_r[:, :, :, :])
            eb16 = cpool.tile([BS, OUT], F32)
            nc.scalar.dma_start(out=eb16, in_=eb_d[:, :])
            rb16 = cpool.tile([BS, OUT], F32)
            nc.scalar.dma_start(out=rb16, in_=rb_d[:, :])
            mb16 = cpool.tile([BS, OUT], F32)
            nc.scalar.dma_start(out=mb16, in_=mb_d[:, :])
            nc.scalar.activation(rb16, rb16, AF.Exp)
            nc.scalar.activation(rb16, rb16, AF.Ln, bias=1.0)
            psmu = pmupool.tile([BS, OUT], F32)
            b16r = cpool.tile([BS, OUT], F32R)

            def emit_bias_math():
                """x @ mu + combined bias row; emitted after sample 2's
                stream work so it does not head-block DVE/PE at startup."""
                for cb in range(CPP):
                    for nh in range(2):
                        nc.tensor.matmul(
                            psmu[:, nh * 512 : (nh + 1) * 512],
                            xt[:, cb * BS : (cb + 1) * BS],
                            mt[:, cb * OUT + nh * 512 : cb * OUT + (nh + 1) * 512],
                            start=(cb == 0),
                            stop=(cb == CPP - 1),
                        )
                nc.vector.tensor_mul(out=eb16, in0=eb16, in1=rb16)
                nc.vector.tensor_add(out=eb16, in0=eb16, in1=mb16)
                nc.vector.tensor_add(out=b16r, in0=eb16, in1=psmu)

            def sample_compute(b, ep, base):
                """multiply + matmuls for sample b whose eps data lives at
                ep[:, base : base+FREE]; bias/copy/store in finish_sample."""
                ps = ppool.tile([1, OUT], F32)
                for h in range(2):
                    epr = eprpool.tile([P, HREE], F16, tag="epr")
                    nc.vector.tensor_mul(
                        out=epr,
                        in0=ep[:, base + h * HREE : base + (h + 1) * HREE],
                        in1=sig[:, h * HREE : (h + 1) * HREE],
                    )
                    for c2 in range(2):
                        cb = 2 * h + c2
                        col = xt[:, cb * BS + b : cb * BS + b + 1]
                        for nh in range(2):
                            nc.tensor.matmul(
                                ps[0:1, nh * 512 : (nh + 1) * 512],
                                col,
                                epr[:, c2 * OUT + nh * 512 : c2 * OUT + (nh + 1) * 512],
                                start=(h == 0 and c2 == 0),
                                stop=False,
                            )
                return ps

            def finish_sample(b, ps):
                for nh in range(2):
                    nc.tensor.matmul(
                        ps[0:1, nh * 512 : (nh + 1) * 512],
                        idr[:, b : b + 1],
                        b16r[:, nh * 512 : (nh + 1) * 512],
                        start=False,
                        stop=True,
                    )
                orow = spool.tile([1, OUT], F32)
                nc.scalar.copy(orow, ps[0:1, :])
                nc.gpsimd.dma_start(out=out_d[b : b + 1, :], in_=orow)

            # ---- main stream: 15 two-sample tiles on the sync ring ------
            pending = []  # (b, ps) awaiting bias math
            for t in range(BS // 2 - 1):
                b0 = 2 * t
                ep = epool.tile([P, 2 * FREE], F16, tag="ep")
                eps_src = eps_d[b0 : b0 + 2, :, :].rearrange(
                    "s (c2 p c1) o -> p s c2 c1 o", p=P, c1=2
                )
                nc.sync.dma_start(out=ep, in_=eps_src)
                for s in range(2):
                    b = b0 + s
                    if b == 3:
                        # flush BEFORE sample 3 allocates PSUM buf 0, which
                        # is only freed by sample 0's bias+copy
                        emit_bias_math()
                        for pb, pps in pending:
                            finish_sample(pb, pps)
                        pending = []
                    ps = sample_compute(b, ep, s * FREE)
                    if b < 3:
                        pending.append((b, ps))
                    else:
                        finish_sample(b, ps)

            # ---- sample 30: single tile ---------------------------------
            b = BS - 2
            ep = lpool.tile([P, FREE], F16)
            eps_src = eps_d[b, :, :].rearrange("(c2 p c1) o -> p c2 c1 o", p=P, c1=2)
            nc.sync.dma_start(out=ep, in_=eps_src[:, :, :, :])
            finish_sample(b, sample_compute(b, ep, 0))

            # ---- sample 31: quarter tiles to shrink the tail ------------
            b = BS - 1
            eps_src = eps_d[b, :, :].rearrange("(c2 p c1) o -> p c2 c1 o", p=P, c1=2)
            ps = ppool.tile([1, OUT], F32)
            ep = lpool.tile([P, FREE], F16)
            for cb in range(CPP):
                nc.sync.dma_start(
                    out=ep[:, cb * OUT : (cb + 1) * OUT],
                    in_=eps_src[:, cb // 2 : cb // 2 + 1, cb % 2 : cb % 2 + 1, :],
                )
                epr = eprpool.tile([P, HREE], F16, tag="epr")
                nc.vector.tensor_mul(
                    out=epr[:, :OUT],
                    in0=ep[:, cb * OUT : (cb + 1) * OUT],
                    in1=sig[:, cb * OUT : (cb + 1) * OUT],
                )
                col = xt[:, cb * BS + b : cb * BS + b + 1]
                for nh in range(2):
                    nc.tensor.matmul(
                        ps[0:1, nh * 512 : (nh + 1) * 512],
                        col,
                        epr[:, nh * 512 : (nh + 1) * 512],
                        start=(cb == 0),
                        stop=False,
                    )
            finish_sample(b, ps)

    nc.finalize()
    return nc


_NC_CACHE = None


def _get_nc():
    global _NC_CACHE
    if _NC_CACHE is None:
        _NC_CACHE = build_nc()
    return _NC_CACHE


def kernel(x, mu, ro, mu_bias, ro_bias, eps, eps_bias, _trace=False, _tmpdir=None):
    x = np.ascontiguousarray(np.asarray(x, dtype=np.float32))
    mu = np.asarray(mu, dtype=np.float32).astype(np.float16)
    ro = np.asarray(ro, dtype=np.float32).astype(np.float16)
    mu_bias = np.asarray(mu_bias, dtype=np.float32).reshape(1, OUT)
    ro_bias = np.asarray(ro_bias, dtype=np.float32).reshape(1, OUT)
    eps = np.asarray(eps, dtype=np.float32)
    eps_bias = np.ascontiguousarray(np.asarray(eps_bias, dtype=np.float32))

    nc = _get_nc()

    zeros_bs = np.zeros((BS, OUT), dtype=np.float32)
    rb_full = np.ascontiguousarray(np.broadcast_to(ro_bias, (BS, OUT)))
    mb_full = np.ascontiguousarray(np.broadcast_to(mu_bias, (BS, OUT)))

    in_maps = []
    for core in range(NCORES):
        g, j = core // ISH, core % ISH
        b0, b1 = g * BS, (g + 1) * BS
        i0, i1 = j * INS, (j + 1) * INS
        # xt[p, cb*BS + b] = x[b, i0 + c2*256 + 2p + c1], cb = 2*c2 + c1
        xt = np.ascontiguousarray(
            x[b0:b1, i0:i1]
            .reshape(BS, 2, P, 2)
            .transpose(2, 1, 3, 0)
            .reshape(P, CPP * BS)
        ).astype(np.float16)
        in_maps.append(
            {
                "eps": np.ascontiguousarray(eps[b0:b1, i0:i1, :]).astype(np.float16),
                "ro": np.ascontiguousarray(ro[i0:i1, :]),
                "mu": np.ascontiguousarray(mu[i0:i1, :]),
                "xt": xt,
                "eps_bias": eps_bias[b0:b1] if j == 0 else zeros_bs,
                "ro_bias": rb_full,
                "mu_bias": mb_full if j == 0 else zeros_bs,
            }
        )

    res = run_bass_kernel_spmd(
        nc, in_maps, core_ids=list(range(NCORES)), trace=_trace, tmpdir=_tmpdir
    )
    out = np.empty((B, OUT), dtype=np.float32)
    for g in range(BG):
        acc = res.results[g * ISH]["out"].copy()
        for j in range(1, ISH):
            acc += res.results[g * ISH + j]["out"]
        out[g * BS : (g + 1) * BS] = acc
    if _trace:
        kernel.last_results = res
    return out
